# revision 1
# baseline (speedup 1.0000x reference)
"""Tensor-parallel LLaMA attention (B=1, S=2048, HID=4096, 32 Q heads / 8 KV
heads, HD=128) on 8 TRN2 NeuronCores.

Sharding: core c owns Q heads [4c..4c+3] and KV head c (column-parallel
q/k/v_proj, row-parallel o_proj). Each core emits a partial [S, HID] output;
the host sums the 8 partials (the all-reduce of the row-parallel o_proj).

Per-core kernel layout strategy (everything [partition, free]):
  - qT/kT produced directly in [d, s] layout (weights as matmul stationary),
    RoPE applied in that layout via partition-offset reads.
  - v produced in natural [s, d] layout with a ones column appended ([s, 129])
    so the ctx matmul's extra column accumulates the softmax row-sum for free.
  - scores computed transposed sT[j, i] = k @ q^T; softmax = exp (no max pass:
    inputs are unit-variance so scores are O(1)); normalization deferred to a
    per-partition scalar multiply after the ctx matmul.
  - ctx[i, 129] -> normalize -> PE-transpose -> ctxT[d, i] -> o_proj.
Causal masking is structural (upper-triangle blocks skipped; diagonal blocks
get a precomputed staircase mask added pre-exp).
"""

import math
import numpy as np
from ml_dtypes import bfloat16

import concourse.bass as bass
import concourse.bacc as bacc
import concourse.tile as tile
import concourse.mybir as mybir
from concourse.bass_utils import run_bass_kernel_spmd

F32 = mybir.dt.float32
BF16 = mybir.dt.bfloat16
AF = mybir.ActivationFunctionType

B, S, HID = 1, 2048, 4096
NH, NKV, HD = 32, 8, 128
NCORES = 8
QH = NH // NCORES          # 4 q heads per core
DQ = QH * HD               # 512
KC = HID // 128            # 32 contraction chunks
NT = S // 128              # 16 s-tiles
NB = S // 512              # 4 s-blocks
THETA = 10000.0
SCALE = 1.0 / math.sqrt(HD)
NEG = -1.0e9


def build_program(mask_mode: str):
    """mask_mode: 'causal' | 'none' | 'full'"""
    nc = bacc.Bacc("TRN2", target_bir_lowering=False, debug=False,
                   enable_asserts=False, num_devices=NCORES)

    hT = nc.dram_tensor("hT", [HID, S], BF16, kind="ExternalInput")
    wq = nc.dram_tensor("wq", [HID, DQ], BF16, kind="ExternalInput")
    wk = nc.dram_tensor("wk", [HID, HD], BF16, kind="ExternalInput")
    wv = nc.dram_tensor("wv", [HID, HD], BF16, kind="ExternalInput")
    wo = nc.dram_tensor("wo", [DQ, HID], BF16, kind="ExternalInput")
    cs = nc.dram_tensor("cs", [2, HD, S], F32, kind="ExternalInput")
    idm = nc.dram_tensor("idm", [HD, HD], F32, kind="ExternalInput")
    if mask_mode == "causal":
        stair = nc.dram_tensor("stair", [4, HD, 512], F32, kind="ExternalInput")
    if mask_mode == "full":
        maskT = nc.dram_tensor("maskT", [S, S], F32, kind="ExternalInput")
    out = nc.dram_tensor("out", [S, HID], F32, kind="ExternalOutput")

    hT_r = hT.rearrange("(c p) s -> p c s", p=128)     # [128, 32, 2048]
    wq_r = wq.rearrange("(c p) m -> p c m", p=128)     # [128, 32, 512]
    wk_r = wk.rearrange("(c p) m -> p c m", p=128)
    wv_r = wv.rearrange("(c p) m -> p c m", p=128)
    wo_r = wo.rearrange("(c p) n -> p c n", p=128)     # [128, 4, 4096]

    with tile.TileContext(nc) as tc:
        with tc.tile_pool(name="persist", bufs=1) as pers:
            qT4 = pers.tile([128, QH, S], BF16)
            kT = pers.tile([128, S], BF16)
            vh = pers.tile([128, NT, 132], BF16)
            ctxT = pers.tile([128, QH, S], BF16)
            id_sb = pers.tile([128, 128], F32)
            nc.sync.dma_start(id_sb[:], idm[:])
            if mask_mode == "causal":
                stair_sb = pers.tile([128, 4, 512], F32)
                nc.sync.dma_start(stair_sb[:], stair[:].rearrange("r p f -> p r f"))

            # ---------------- Phase 1: QKV projection + RoPE ----------------
            with tc.tile_pool(name="ph1", bufs=1) as p1, \
                 tc.tile_pool(name="ph1d", bufs=2) as p1d, \
                 tc.tile_pool(name="pp1", bufs=2, space="PSUM") as pp1:
                wq_sb = p1.tile([128, KC, DQ], BF16)
                nc.sync.dma_start(wq_sb[:], wq_r[:])
                wk_sb = p1.tile([128, KC, HD], BF16)
                nc.sync.dma_start(wk_sb[:], wk_r[:])
                wv_sb = p1.tile([128, KC, HD], BF16)
                nc.sync.dma_start(wv_sb[:], wv_r[:])
                cos_sb = p1.tile([128, S], F32)
                nc.sync.dma_start(cos_sb[:], cs[0])
                sin_sb = p1.tile([128, S], F32)
                nc.sync.dma_start(sin_sb[:], cs[1])

                for st in range(NT):
                    nc.vector.memset(vh[:, st, 128:129], 1.0)

                for sb in range(NB):
                    sl = slice(sb * 512, (sb + 1) * 512)
                    ht = p1d.tile([128, KC, 512], BF16, tag="ht")
                    nc.sync.dma_start(ht[:], hT_r[:, :, sl])
                    # q (4 head-tiles) then k
                    for hti in range(QH + 1):
                        ps = pp1.tile([128, 512], F32, tag="ps1")
                        for c in range(KC):
                            lhsT = (wq_sb[:, c, hti * 128:(hti + 1) * 128]
                                    if hti < QH else wk_sb[:, c, :])
                            nc.tensor.matmul(ps[:], lhsT, ht[:, c, :],
                                             start=(c == 0), stop=(c == KC - 1))
                        # RoPE in [d, s] layout
                        tc_ = p1d.tile([128, 512], F32, tag="tcos")
                        ts_ = p1d.tile([128, 512], F32, tag="tsin")
                        nc.vector.tensor_mul(tc_[:], ps[:], cos_sb[:, sl])
                        nc.vector.tensor_mul(ts_[0:64, :], ps[64:128, :],
                                             sin_sb[0:64, sl])
                        nc.vector.tensor_mul(ts_[64:128, :], ps[0:64, :],
                                             sin_sb[64:128, sl])
                        dest = (qT4[:, hti, sl] if hti < QH else kT[:, sl])
                        nc.vector.tensor_add(dest, tc_[:], ts_[:])
                    # v in [s, d] layout (+ ones col already set)
                    for st4 in range(4):
                        st = sb * 4 + st4
                        psv = pp1.tile([128, 128], F32, tag="psv")
                        for c in range(KC):
                            nc.tensor.matmul(psv[:],
                                             ht[:, c, st4 * 128:(st4 + 1) * 128],
                                             wv_sb[:, c, :],
                                             start=(c == 0), stop=(c == KC - 1))
                        nc.scalar.activation(vh[:, st, 0:128], psv[:], AF.Copy)

            # ---------------- Phase 2: attention ----------------
            with tc.tile_pool(name="ph2", bufs=2) as p2, \
                 tc.tile_pool(name="pp2", bufs=3, space="PSUM") as pp2, \
                 tc.tile_pool(name="pp2c", bufs=2, space="PSUM") as pp2c:
                for h in range(QH):
                    for ib in range(NB):
                        isl = slice(ib * 512, (ib + 1) * 512)
                        jmax = 4 * (ib + 1) if mask_mode == "causal" else NT
                        pT = p2.tile([128, NT, 512], BF16, tag="pT")
                        for jt in range(jmax):
                            pss = pp2.tile([128, 512], F32, tag="pss")
                            nc.tensor.matmul(pss[:],
                                             kT[:, jt * 128:(jt + 1) * 128],
                                             qT4[:, h, isl],
                                             start=True, stop=True)
                            if mask_mode == "causal" and jt >= 4 * ib:
                                r = jt - 4 * ib
                                nc.vector.tensor_add(pss[:], pss[:],
                                                     stair_sb[:, r, :])
                            elif mask_mode == "full":
                                mt = p2.tile([128, 512], F32, tag="mt")
                                nc.sync.dma_start(
                                    mt[:], maskT[jt * 128:(jt + 1) * 128, isl])
                                nc.vector.tensor_add(pss[:], pss[:], mt[:])
                            nc.scalar.activation(pT[:, jt, :], pss[:], AF.Exp,
                                                 scale=SCALE)
                        for it in range(4):
                            ig = ib * 4 + it
                            jm = ig + 1 if mask_mode == "causal" else NT
                            psc = pp2c.tile([128, 132], F32, tag="psc")
                            for jt in range(jm):
                                nc.tensor.matmul(
                                    psc[:, 0:129],
                                    pT[:, jt, it * 128:(it + 1) * 128],
                                    vh[:, jt, 0:129],
                                    start=(jt == 0), stop=(jt == jm - 1))
                            rec = p2.tile([128, 1], F32, tag="rec")
                            nc.vector.reciprocal(rec[:], psc[:, 128:129])
                            cn = p2.tile([128, 128], F32, tag="cn")
                            nc.vector.tensor_scalar_mul(cn[:], psc[:, 0:128],
                                                        rec[:])
                            pst = pp2c.tile([128, 128], F32, tag="pst")
                            nc.tensor.transpose(pst[:], cn[:], id_sb[:])
                            nc.scalar.activation(
                                ctxT[:, h, ig * 128:(ig + 1) * 128],
                                pst[:], AF.Copy)

            # ---------------- Phase 3: O projection (row-parallel partial) ---
            with tc.tile_pool(name="ph3", bufs=1) as p3, \
                 tc.tile_pool(name="ph3d", bufs=4) as p3d, \
                 tc.tile_pool(name="pp3", bufs=4, space="PSUM") as pp3:
                wo_sb = p3.tile([128, QH, HID], BF16)
                nc.sync.dma_start(wo_sb[:], wo_r[:])
                for it in range(NT):
                    for nb in range(HID // 512):
                        pso = pp3.tile([128, 512], F32, tag="pso")
                        for c4 in range(QH):
                            nc.tensor.matmul(
                                pso[:],
                                ctxT[:, c4, it * 128:(it + 1) * 128],
                                wo_sb[:, c4, nb * 512:(nb + 1) * 512],
                                start=(c4 == 0), stop=(c4 == QH - 1))
                        o_sb = p3d.tile([128, 512], F32, tag="osb")
                        nc.vector.tensor_copy(o_sb[:], pso[:])
                        nc.sync.dma_start(
                            out[it * 128:(it + 1) * 128,
                                nb * 512:(nb + 1) * 512], o_sb[:])

    nc.compile()
    return nc


_CACHE: dict = {}


def _get_program(mask_mode: str):
    if mask_mode not in _CACHE:
        _CACHE[mask_mode] = build_program(mask_mode)
    return _CACHE[mask_mode]


def _host_tensors():
    """Position-dependent constants shared by every call."""
    inv_freq = 1.0 / (THETA ** (np.arange(0, HD, 2, dtype=np.float32) / HD))
    t = np.arange(S, dtype=np.float32)
    freqs = np.outer(t, inv_freq)                     # [S, 64]
    emb = np.concatenate([freqs, freqs], axis=-1)     # [S, 128]
    cosT = np.cos(emb).T.astype(np.float32).copy()    # [128, S]
    sinT = np.sin(emb).T.astype(np.float32).copy()
    sinT[0:64] *= -1.0                                # fold rotate_half sign
    cs = np.ascontiguousarray(np.stack([cosT, sinT]))  # [2, 128, S]
    idm = np.eye(128, dtype=np.float32)
    jj = np.arange(128)[:, None]
    ii = np.arange(512)[None, :]
    stair = np.stack([np.where(ii >= 128 * r + jj, 0.0, NEG)
                      for r in range(4)]).astype(np.float32)  # [4, 128, 512]
    return cs, idm, stair


def kernel(hidden_states, Wq, Wk, Wv, Wo, attention_mask):
    hidden_states = np.asarray(hidden_states, dtype=np.float32)
    Wq = np.asarray(Wq, dtype=np.float32)
    Wk = np.asarray(Wk, dtype=np.float32)
    Wv = np.asarray(Wv, dtype=np.float32)
    Wo = np.asarray(Wo, dtype=np.float32)
    mask = np.asarray(attention_mask, dtype=np.float32)[0, 0]

    causal_ref = np.triu(np.full((S, S), NEG, dtype=np.float32), k=1)
    if np.array_equal(mask, causal_ref):
        mode = "causal"
    elif not mask.any():
        mode = "none"
    else:
        mode = "full"

    nc = _get_program(mode)
    cs, idm, stair = _host_tensors()

    hT = np.ascontiguousarray(hidden_states[0].T).astype(bfloat16)
    wq_b = Wq.astype(bfloat16)
    wk_b = Wk.astype(bfloat16)
    wv_b = Wv.astype(bfloat16)
    wo_b = Wo.astype(bfloat16)

    in_maps = []
    for c in range(NCORES):
        m = {
            "hT": hT,
            "wq": np.ascontiguousarray(wq_b[:, c * DQ:(c + 1) * DQ]),
            "wk": np.ascontiguousarray(wk_b[:, c * HD:(c + 1) * HD]),
            "wv": np.ascontiguousarray(wv_b[:, c * HD:(c + 1) * HD]),
            "wo": np.ascontiguousarray(wo_b[c * DQ:(c + 1) * DQ, :]),
            "cs": cs,
            "idm": idm,
        }
        if mode == "causal":
            m["stair"] = stair
        if mode == "full":
            m["maskT"] = np.ascontiguousarray(mask.T * math.sqrt(HD)).astype(
                np.float32)
        in_maps.append(m)

    res = run_bass_kernel_spmd(nc, in_maps, core_ids=list(range(NCORES)))
    total = res.results[0]["out"].astype(np.float32)
    for c in range(1, NCORES):
        total = total + res.results[c]["out"]
    return total.reshape(B, S, HID).astype(np.float32)


# revision 8
# speedup vs baseline: 1.1155x; 1.1155x over previous
"""Tensor-parallel LLaMA attention (B=1, S=2048, HID=4096, 32 Q heads / 8 KV
heads, HD=128) on 8 TRN2 NeuronCores.

Sharding: core c owns Q heads [4c..4c+3] and KV head c (column-parallel
q/k/v_proj, row-parallel o_proj). Each core emits a partial [S, HID] output;
the host sums the 8 partials (the all-reduce of the row-parallel o_proj).

Per-core kernel layout strategy (everything [partition, free]):
  - qT/kT produced directly in [d, s] layout (weights as matmul stationary),
    RoPE applied in that layout via partition-offset reads.
  - v produced in natural [s, d] layout with a ones column appended ([s, 129])
    so the ctx matmul's extra column accumulates the softmax row-sum for free.
  - scores computed transposed sT[j, i] = k @ q^T; softmax = exp (no max pass:
    inputs are unit-variance so scores are O(1)); normalization deferred to a
    per-partition scalar multiply after the ctx matmul.
  - ctx[i, 129] -> normalize -> PE-transpose -> ctxT[d, i] -> o_proj.
Causal masking is structural (upper-triangle blocks skipped; diagonal blocks
get a precomputed staircase mask added pre-exp).
"""

import math
import numpy as np
from ml_dtypes import bfloat16

import concourse.bass as bass
import concourse.bacc as bacc
import concourse.tile as tile
import concourse.mybir as mybir
from concourse.bass_utils import run_bass_kernel_spmd

F32 = mybir.dt.float32
BF16 = mybir.dt.bfloat16
AF = mybir.ActivationFunctionType

B, S, HID = 1, 2048, 4096
NH, NKV, HD = 32, 8, 128
NCORES = 8
QH = NH // NCORES          # 4 q heads per core
DQ = QH * HD               # 512
KC = HID // 128            # 32 contraction chunks
NT = S // 128              # 16 s-tiles
NB = S // 512              # 4 s-blocks
THETA = 10000.0
SCALE = 1.0 / math.sqrt(HD)
NEG = -1.0e9


def build_program(mask_mode: str):
    """mask_mode: 'causal' | 'none' | 'full'"""
    nc = bacc.Bacc("TRN2", target_bir_lowering=False, debug=False,
                   enable_asserts=False, num_devices=NCORES)

    hT = nc.dram_tensor("hT", [HID, S], BF16, kind="ExternalInput")
    wq = nc.dram_tensor("wq", [HID, DQ], BF16, kind="ExternalInput")
    wk = nc.dram_tensor("wk", [HID, HD], BF16, kind="ExternalInput")
    wv = nc.dram_tensor("wv", [HID, HD], BF16, kind="ExternalInput")
    wo = nc.dram_tensor("wo", [DQ, HID], BF16, kind="ExternalInput")
    cs = nc.dram_tensor("cs", [2, HD, S], F32, kind="ExternalInput")
    idm = nc.dram_tensor("idm", [HD, HD], F32, kind="ExternalInput")
    if mask_mode == "causal":
        stair = nc.dram_tensor("stair", [4, HD, 512], BF16, kind="ExternalInput")
    if mask_mode == "full":
        maskT = nc.dram_tensor("maskT", [S, S], F32, kind="ExternalInput")
    out = nc.dram_tensor("out", [S, HID], F32, kind="ExternalOutput")

    hT_r = hT.rearrange("(c p) s -> p c s", p=128)     # [128, 32, 2048]
    wq_r = wq.rearrange("(c p) m -> p c m", p=128)     # [128, 32, 512]
    wk_r = wk.rearrange("(c p) m -> p c m", p=128)
    wv_r = wv.rearrange("(c p) m -> p c m", p=128)
    wo_r = wo.rearrange("(c p) n -> p c n", p=128)     # [128, 4, 4096]

    with tile.TileContext(nc) as tc:
        with tc.tile_pool(name="persist", bufs=1) as pers:
            qT4 = pers.tile([128, QH, S], BF16)
            kT = pers.tile([128, S], BF16)
            vh = pers.tile([128, NT, 132], BF16)
            ctxT = pers.tile([128, QH, S], BF16)
            id_sb = pers.tile([128, 128], F32)
            nc.sync.dma_start(id_sb[:], idm[:])
            if mask_mode == "causal":
                stair_sb = pers.tile([128, 4, 512], BF16)
                nc.sync.dma_start(stair_sb[:], stair[:].rearrange("r p f -> p r f"))

            # ---------------- Phase 1: QKV projection + RoPE ----------------
            with tc.tile_pool(name="ph1", bufs=1) as p1, \
                 tc.tile_pool(name="ph1d", bufs=2) as p1d, \
                 tc.tile_pool(name="pp1", bufs=2, space="PSUM") as pp1:
                wq_sb = p1.tile([128, KC, DQ], BF16)
                nc.sync.dma_start(wq_sb[:, 0:8, :], wq_r[:, 0:8, :])
                nc.sync.dma_start(wq_sb[:, 8:16, :], wq_r[:, 8:16, :])
                nc.sync.dma_start(wq_sb[:, 16:24, :], wq_r[:, 16:24, :])
                nc.sync.dma_start(wq_sb[:, 24:32, :], wq_r[:, 24:32, :])
                wk_sb = p1.tile([128, KC, HD], BF16)
                nc.sync.dma_start(wk_sb[:], wk_r[:])
                wv_sb = p1.tile([128, KC, HD], BF16)
                nc.sync.dma_start(wv_sb[:], wv_r[:])
                cos_sb = p1.tile([128, S], F32)
                nc.sync.dma_start(cos_sb[:], cs[0])
                sin_sb = p1.tile([128, S], F32)
                nc.sync.dma_start(sin_sb[:], cs[1])

                for st in range(NT):
                    nc.vector.memset(vh[:, st, 128:129], 1.0)

                for sb in range(NB):
                    sl = slice(sb * 512, (sb + 1) * 512)
                    ht = p1d.tile([128, KC, 512], BF16, tag="ht")
                    nc.sync.dma_start(ht[:, 0:8, :], hT_r[:, 0:8, sl])
                    nc.sync.dma_start(ht[:, 8:16, :], hT_r[:, 8:16, sl])
                    nc.sync.dma_start(ht[:, 16:24, :], hT_r[:, 16:24, sl])
                    nc.sync.dma_start(ht[:, 24:32, :], hT_r[:, 24:32, sl])
                    # q (4 head-tiles) then k
                    for hti in range(QH + 1):
                        ps = pp1.tile([128, 512], F32, tag="ps1")
                        for c in range(KC):
                            lhsT = (wq_sb[:, c, hti * 128:(hti + 1) * 128]
                                    if hti < QH else wk_sb[:, c, :])
                            nc.tensor.matmul(ps[:], lhsT, ht[:, c, :],
                                             start=(c == 0), stop=(c == KC - 1))
                        # RoPE in [d, s] layout
                        tc_ = p1d.tile([128, 512], F32, tag="tcos")
                        ts_ = p1d.tile([128, 512], F32, tag="tsin")
                        nc.vector.tensor_mul(tc_[:], ps[:], cos_sb[:, sl])
                        nc.vector.tensor_mul(ts_[0:64, :], ps[64:128, :],
                                             sin_sb[0:64, sl])
                        nc.vector.tensor_mul(ts_[64:128, :], ps[0:64, :],
                                             sin_sb[64:128, sl])
                        dest = (qT4[:, hti, sl] if hti < QH else kT[:, sl])
                        nc.vector.tensor_add(dest, tc_[:], ts_[:])
                    # v in [s, d] layout (+ ones col already set)
                    for st4 in range(4):
                        st = sb * 4 + st4
                        psv = pp1.tile([128, 128], F32, tag="psv")
                        for c in range(KC):
                            nc.tensor.matmul(psv[:],
                                             ht[:, c, st4 * 128:(st4 + 1) * 128],
                                             wv_sb[:, c, :],
                                             start=(c == 0), stop=(c == KC - 1))
                        nc.scalar.activation(vh[:, st, 0:128], psv[:], AF.Copy)

            # ------- Phase 2+3: attention with interleaved O projection ------
            # i-block outer so o_proj for block ib (needing all 4 heads'
            # ctxT columns) interleaves with attention of block ib+1,
            # keeping PE fed through the ACT/DVE softmax chains.
            with tc.tile_pool(name="ph2", bufs=2) as p2, \
                 tc.tile_pool(name="ph3", bufs=1) as p3, \
                 tc.tile_pool(name="ph3d", bufs=4) as p3d, \
                 tc.tile_pool(name="pp2", bufs=3, space="PSUM") as pp2, \
                 tc.tile_pool(name="pp2c", bufs=2, space="PSUM") as pp2c, \
                 tc.tile_pool(name="pp3", bufs=2, space="PSUM") as pp3:
                wo_sb = p3.tile([128, QH, HID], BF16)
                nc.sync.dma_start(wo_sb[:, 0:2, :], wo_r[:, 0:2, :])
                nc.sync.dma_start(wo_sb[:, 2:4, :], wo_r[:, 2:4, :])
                for ib in range(NB):
                    isl = slice(ib * 512, (ib + 1) * 512)
                    jmax = 4 * (ib + 1) if mask_mode == "causal" else NT
                    for h in range(QH):
                        pT = p2.tile([128, NT, 512], BF16, tag="pT")
                        for jt in range(jmax):
                            pss = pp2.tile([128, 512], F32, tag="pss")
                            nc.tensor.matmul(pss[:],
                                             kT[:, jt * 128:(jt + 1) * 128],
                                             qT4[:, h, isl],
                                             start=True, stop=True)
                            if mask_mode == "full":
                                mt = p2.tile([128, 512], F32, tag="mt")
                                nc.sync.dma_start(
                                    mt[:], maskT[jt * 128:(jt + 1) * 128, isl])
                                nc.vector.tensor_add(pss[:], pss[:], mt[:])
                            nc.scalar.activation(pT[:, jt, :], pss[:], AF.Exp,
                                                 scale=SCALE)
                            if mask_mode == "causal" and jt >= 4 * ib:
                                # zero the upper triangle post-exp (0/1 mask,
                                # bf16 SBUF in-place: 4x DVE rate, off PSUM)
                                r = jt - 4 * ib
                                nc.vector.tensor_mul(pT[:, jt, :],
                                                     pT[:, jt, :],
                                                     stair_sb[:, r, :])
                        for it in range(4):
                            ig = ib * 4 + it
                            jm = ig + 1 if mask_mode == "causal" else NT
                            psc = pp2c.tile([128, 132], F32, tag="psc")
                            for jt in range(jm):
                                nc.tensor.matmul(
                                    psc[:, 0:129],
                                    pT[:, jt, it * 128:(it + 1) * 128],
                                    vh[:, jt, 0:129],
                                    start=(jt == 0), stop=(jt == jm - 1))
                            rec = p2.tile([128, 1], F32, tag="rec")
                            nc.vector.reciprocal(rec[:], psc[:, 128:129])
                            cn = p2.tile([128, 128], F32, tag="cn")
                            nc.vector.tensor_scalar_mul(cn[:], psc[:, 0:128],
                                                        rec[:])
                            pst = pp2c.tile([128, 128], F32, tag="pst", bufs=1)
                            nc.tensor.transpose(pst[:], cn[:], id_sb[:])
                            nc.scalar.activation(
                                ctxT[:, h, ig * 128:(ig + 1) * 128],
                                pst[:], AF.Copy)
                    # o_proj for this i-block (all 4 heads ready)
                    for it in range(4):
                        itg = ib * 4 + it
                        for nb in range(HID // 512):
                            pso = pp3.tile([128, 512], F32, tag="pso")
                            for c4 in range(QH):
                                nc.tensor.matmul(
                                    pso[:],
                                    ctxT[:, c4, itg * 128:(itg + 1) * 128],
                                    wo_sb[:, c4, nb * 512:(nb + 1) * 512],
                                    start=(c4 == 0), stop=(c4 == QH - 1))
                            o_sb = p3d.tile([128, 512], F32, tag="osb")
                            if nb % 2 == 0:
                                nc.vector.tensor_copy(o_sb[:], pso[:])
                            else:
                                nc.scalar.activation(o_sb[:], pso[:], AF.Copy)
                            nc.sync.dma_start(
                                out[itg * 128:(itg + 1) * 128,
                                    nb * 512:(nb + 1) * 512], o_sb[:])

    nc.compile()
    return nc


_CACHE: dict = {}


def _get_program(mask_mode: str):
    if mask_mode not in _CACHE:
        _CACHE[mask_mode] = build_program(mask_mode)
    return _CACHE[mask_mode]


def _host_tensors():
    """Position-dependent constants shared by every call."""
    inv_freq = 1.0 / (THETA ** (np.arange(0, HD, 2, dtype=np.float32) / HD))
    t = np.arange(S, dtype=np.float32)
    freqs = np.outer(t, inv_freq)                     # [S, 64]
    emb = np.concatenate([freqs, freqs], axis=-1)     # [S, 128]
    cosT = np.cos(emb).T.astype(np.float32).copy()    # [128, S]
    sinT = np.sin(emb).T.astype(np.float32).copy()
    sinT[0:64] *= -1.0                                # fold rotate_half sign
    cs = np.ascontiguousarray(np.stack([cosT, sinT]))  # [2, 128, S]
    idm = np.eye(128, dtype=np.float32)
    jj = np.arange(128)[:, None]
    ii = np.arange(512)[None, :]
    stair = np.stack([np.where(ii >= 128 * r + jj, 1.0, 0.0)
                      for r in range(4)]).astype(bfloat16)  # [4, 128, 512]
    return cs, idm, stair


def kernel(hidden_states, Wq, Wk, Wv, Wo, attention_mask):
    hidden_states = np.asarray(hidden_states, dtype=np.float32)
    Wq = np.asarray(Wq, dtype=np.float32)
    Wk = np.asarray(Wk, dtype=np.float32)
    Wv = np.asarray(Wv, dtype=np.float32)
    Wo = np.asarray(Wo, dtype=np.float32)
    mask = np.asarray(attention_mask, dtype=np.float32)[0, 0]

    causal_ref = np.triu(np.full((S, S), NEG, dtype=np.float32), k=1)
    if np.array_equal(mask, causal_ref):
        mode = "causal"
    elif not mask.any():
        mode = "none"
    else:
        mode = "full"

    nc = _get_program(mode)
    cs, idm, stair = _host_tensors()

    hT = np.ascontiguousarray(hidden_states[0].T).astype(bfloat16)
    wq_b = Wq.astype(bfloat16)
    wk_b = Wk.astype(bfloat16)
    wv_b = Wv.astype(bfloat16)
    wo_b = Wo.astype(bfloat16)

    in_maps = []
    for c in range(NCORES):
        m = {
            "hT": hT,
            "wq": np.ascontiguousarray(wq_b[:, c * DQ:(c + 1) * DQ]),
            "wk": np.ascontiguousarray(wk_b[:, c * HD:(c + 1) * HD]),
            "wv": np.ascontiguousarray(wv_b[:, c * HD:(c + 1) * HD]),
            "wo": np.ascontiguousarray(wo_b[c * DQ:(c + 1) * DQ, :]),
            "cs": cs,
            "idm": idm,
        }
        if mode == "causal":
            m["stair"] = stair
        if mode == "full":
            m["maskT"] = np.ascontiguousarray(mask.T * math.sqrt(HD)).astype(
                np.float32)
        in_maps.append(m)

    res = run_bass_kernel_spmd(nc, in_maps, core_ids=list(range(NCORES)))
    total = res.results[0]["out"].astype(np.float32)
    for c in range(1, NCORES):
        total = total + res.results[c]["out"]
    return total.reshape(B, S, HID).astype(np.float32)


# revision 10
# speedup vs baseline: 1.1672x; 1.0464x over previous
"""Tensor-parallel LLaMA attention (B=1, S=2048, HID=4096, 32 Q heads / 8 KV
heads, HD=128) on 8 TRN2 NeuronCores.

Sharding: core c owns Q heads [4c..4c+3] and KV head c (column-parallel
q/k/v_proj, row-parallel o_proj). Each core emits a partial [S, HID] output;
the host sums the 8 partials (the all-reduce of the row-parallel o_proj).

Per-core kernel layout strategy (everything [partition, free]):
  - qT/kT produced directly in [d, s] layout (weights as matmul stationary),
    RoPE applied in that layout via partition-offset reads.
  - v produced in natural [s, d] layout with a ones column appended ([s, 129])
    so the ctx matmul's extra column accumulates the softmax row-sum for free.
  - scores computed transposed sT[j, i] = k @ q^T; softmax = exp (no max pass:
    inputs are unit-variance so scores are O(1)); normalization deferred to a
    per-partition scalar multiply after the ctx matmul.
  - ctx[i, 129] -> normalize -> PE-transpose -> ctxT[d, i] -> o_proj.
Causal masking is structural (upper-triangle blocks skipped; diagonal blocks
get a precomputed staircase mask added pre-exp).
"""

import math
import numpy as np
from ml_dtypes import bfloat16

import concourse.bass as bass
import concourse.bacc as bacc
import concourse.tile as tile
import concourse.mybir as mybir
from concourse.bass_utils import run_bass_kernel_spmd

F32 = mybir.dt.float32
BF16 = mybir.dt.bfloat16
AF = mybir.ActivationFunctionType

B, S, HID = 1, 2048, 4096
NH, NKV, HD = 32, 8, 128
NCORES = 8
QH = NH // NCORES          # 4 q heads per core
DQ = QH * HD               # 512
KC = HID // 128            # 32 contraction chunks
NT = S // 128              # 16 s-tiles
NB = S // 512              # 4 s-blocks
THETA = 10000.0
SCALE = 1.0 / math.sqrt(HD)
NEG = -1.0e9


def build_program(mask_mode: str):
    """mask_mode: 'causal' | 'none' | 'full'"""
    nc = bacc.Bacc("TRN2", target_bir_lowering=False, debug=False,
                   enable_asserts=False, num_devices=NCORES)

    hT = nc.dram_tensor("hT", [HID, S], BF16, kind="ExternalInput")
    wq = nc.dram_tensor("wq", [HID, DQ], BF16, kind="ExternalInput")
    wk = nc.dram_tensor("wk", [HID, HD], BF16, kind="ExternalInput")
    wv = nc.dram_tensor("wv", [HID, HD], BF16, kind="ExternalInput")
    wo = nc.dram_tensor("wo", [DQ, HID], BF16, kind="ExternalInput")
    cs = nc.dram_tensor("cs", [2, HD, S], F32, kind="ExternalInput")
    idm = nc.dram_tensor("idm", [HD, HD], F32, kind="ExternalInput")
    if mask_mode == "causal":
        stair = nc.dram_tensor("stair", [4, HD, 512], BF16, kind="ExternalInput")
    if mask_mode == "full":
        maskT = nc.dram_tensor("maskT", [S, S], F32, kind="ExternalInput")
    out = nc.dram_tensor("out", [S, HID], F32, kind="ExternalOutput")

    hT_r = hT.rearrange("(c p) s -> p c s", p=128)     # [128, 32, 2048]
    wq_r = wq.rearrange("(c p) m -> p c m", p=128)     # [128, 32, 512]
    wk_r = wk.rearrange("(c p) m -> p c m", p=128)
    wv_r = wv.rearrange("(c p) m -> p c m", p=128)
    wo_r = wo.rearrange("(c p) n -> p c n", p=128)     # [128, 4, 4096]

    with tile.TileContext(nc) as tc:
        with tc.tile_pool(name="persist", bufs=1) as pers:
            qT4 = pers.tile([128, QH, S], BF16)
            kT = pers.tile([128, S], BF16)
            vh = pers.tile([128, NT, 132], BF16)
            ctxT = pers.tile([128, QH, S], BF16)
            id_sb = pers.tile([128, 128], F32)
            nc.sync.dma_start(id_sb[:], idm[:])
            if mask_mode == "causal":
                stair_sb = pers.tile([128, 4, 512], BF16)
                nc.sync.dma_start(stair_sb[:], stair[:].rearrange("r p f -> p r f"))

            # ---------------- Phase 1: QKV projection + RoPE ----------------
            with tc.tile_pool(name="ph1", bufs=1) as p1, \
                 tc.tile_pool(name="ph1d", bufs=2) as p1d, \
                 tc.tile_pool(name="pp1", bufs=2, space="PSUM") as pp1:
                # DMA order = first-needed-first: the opening q-chain reads
                # wq chunk 0 + ht chunk 0, so interleave those streams; k/v
                # weights and cos/sin aren't needed until ~15us in.
                wq_sb = p1.tile([128, KC, DQ], BF16)
                wk_sb = p1.tile([128, KC, HD], BF16)
                wv_sb = p1.tile([128, KC, HD], BF16)
                cos_sb = p1.tile([128, S], F32)
                sin_sb = p1.tile([128, S], F32)
                ht0 = p1d.tile([128, KC, 512], BF16, tag="ht")
                nc.sync.dma_start(wq_sb[:, 0:8, :], wq_r[:, 0:8, :])
                nc.sync.dma_start(ht0[:, 0:8, :], hT_r[:, 0:8, 0:512])
                nc.sync.dma_start(wq_sb[:, 8:16, :], wq_r[:, 8:16, :])
                nc.sync.dma_start(ht0[:, 8:16, :], hT_r[:, 8:16, 0:512])
                nc.sync.dma_start(wq_sb[:, 16:24, :], wq_r[:, 16:24, :])
                nc.sync.dma_start(ht0[:, 16:24, :], hT_r[:, 16:24, 0:512])
                nc.sync.dma_start(wq_sb[:, 24:32, :], wq_r[:, 24:32, :])
                nc.sync.dma_start(ht0[:, 24:32, :], hT_r[:, 24:32, 0:512])
                nc.sync.dma_start(cos_sb[:, 0:512], cs[0, :, 0:512])
                nc.sync.dma_start(sin_sb[:, 0:512], cs[1, :, 0:512])
                nc.sync.dma_start(cos_sb[:, 512:], cs[0, :, 512:])
                nc.sync.dma_start(sin_sb[:, 512:], cs[1, :, 512:])
                nc.sync.dma_start(wk_sb[:], wk_r[:])
                nc.sync.dma_start(wv_sb[:], wv_r[:])

                for st in range(NT):
                    nc.vector.memset(vh[:, st, 128:129], 1.0)

                for sb in range(NB):
                    sl = slice(sb * 512, (sb + 1) * 512)
                    if sb == 0:
                        ht = ht0
                    else:
                        ht = p1d.tile([128, KC, 512], BF16, tag="ht")
                        nc.sync.dma_start(ht[:, 0:8, :], hT_r[:, 0:8, sl])
                        nc.sync.dma_start(ht[:, 8:16, :], hT_r[:, 8:16, sl])
                        nc.sync.dma_start(ht[:, 16:24, :], hT_r[:, 16:24, sl])
                        nc.sync.dma_start(ht[:, 24:32, :], hT_r[:, 24:32, sl])
                    # q (4 head-tiles) then k
                    for hti in range(QH + 1):
                        ps = pp1.tile([128, 512], F32, tag="ps1", bufs=4)
                        for c in range(KC):
                            lhsT = (wq_sb[:, c, hti * 128:(hti + 1) * 128]
                                    if hti < QH else wk_sb[:, c, :])
                            nc.tensor.matmul(ps[:], lhsT, ht[:, c, :],
                                             start=(c == 0), stop=(c == KC - 1))
                        # RoPE in [d, s] layout
                        tc_ = p1d.tile([128, 512], F32, tag="tcos")
                        ts_ = p1d.tile([128, 512], F32, tag="tsin")
                        nc.vector.tensor_mul(tc_[:], ps[:], cos_sb[:, sl])
                        nc.vector.tensor_mul(ts_[0:64, :], ps[64:128, :],
                                             sin_sb[0:64, sl])
                        nc.vector.tensor_mul(ts_[64:128, :], ps[0:64, :],
                                             sin_sb[64:128, sl])
                        dest = (qT4[:, hti, sl] if hti < QH else kT[:, sl])
                        nc.vector.tensor_add(dest, tc_[:], ts_[:])
                    # v in [s, d] layout (+ ones col already set)
                    for st4 in range(4):
                        st = sb * 4 + st4
                        psv = pp1.tile([128, 128], F32, tag="psv")
                        for c in range(KC):
                            nc.tensor.matmul(psv[:],
                                             ht[:, c, st4 * 128:(st4 + 1) * 128],
                                             wv_sb[:, c, :],
                                             start=(c == 0), stop=(c == KC - 1))
                        nc.scalar.activation(vh[:, st, 0:128], psv[:], AF.Copy)

            # ------- Phase 2+3: attention with interleaved O projection ------
            # i-block outer so o_proj for block ib (needing all 4 heads'
            # ctxT columns) interleaves with attention of block ib+1,
            # keeping PE fed through the ACT/DVE softmax chains.
            with tc.tile_pool(name="ph2", bufs=2) as p2, \
                 tc.tile_pool(name="ph3", bufs=1) as p3, \
                 tc.tile_pool(name="ph3d", bufs=4) as p3d, \
                 tc.tile_pool(name="pp2", bufs=3, space="PSUM") as pp2, \
                 tc.tile_pool(name="pp2c", bufs=2, space="PSUM") as pp2c, \
                 tc.tile_pool(name="pp3", bufs=2, space="PSUM") as pp3:
                wo_sb = p3.tile([128, QH, HID], BF16)
                nc.sync.dma_start(wo_sb[:, 0:2, :], wo_r[:, 0:2, :])
                nc.sync.dma_start(wo_sb[:, 2:4, :], wo_r[:, 2:4, :])
                for ib in range(NB):
                    isl = slice(ib * 512, (ib + 1) * 512)
                    jmax = 4 * (ib + 1) if mask_mode == "causal" else NT
                    for h in range(QH):
                        pT = p2.tile([128, NT, 512], BF16, tag="pT")
                        for jt in range(jmax):
                            pss = pp2.tile([128, 512], F32, tag="pss")
                            nc.tensor.matmul(pss[:],
                                             kT[:, jt * 128:(jt + 1) * 128],
                                             qT4[:, h, isl],
                                             start=True, stop=True)
                            if mask_mode == "full":
                                mt = p2.tile([128, 512], F32, tag="mt")
                                nc.sync.dma_start(
                                    mt[:], maskT[jt * 128:(jt + 1) * 128, isl])
                                nc.vector.tensor_add(pss[:], pss[:], mt[:])
                            nc.scalar.activation(pT[:, jt, :], pss[:], AF.Exp,
                                                 scale=SCALE)
                            if mask_mode == "causal" and jt >= 4 * ib:
                                # zero the upper triangle post-exp (0/1 mask,
                                # bf16 SBUF in-place: 4x DVE rate, off PSUM)
                                r = jt - 4 * ib
                                nc.vector.tensor_mul(pT[:, jt, :],
                                                     pT[:, jt, :],
                                                     stair_sb[:, r, :])
                        for it in range(4):
                            ig = ib * 4 + it
                            jm = ig + 1 if mask_mode == "causal" else NT
                            psc = pp2c.tile([128, 132], F32, tag="psc")
                            for jt in range(jm):
                                nc.tensor.matmul(
                                    psc[:, 0:129],
                                    pT[:, jt, it * 128:(it + 1) * 128],
                                    vh[:, jt, 0:129],
                                    start=(jt == 0), stop=(jt == jm - 1))
                            rec = p2.tile([128, 1], F32, tag="rec")
                            nc.vector.reciprocal(rec[:], psc[:, 128:129])
                            cn = p2.tile([128, 128], F32, tag="cn")
                            nc.vector.tensor_scalar_mul(cn[:], psc[:, 0:128],
                                                        rec[:])
                            pst = pp2c.tile([128, 128], F32, tag="pst", bufs=1)
                            nc.tensor.transpose(pst[:], cn[:], id_sb[:])
                            nc.scalar.activation(
                                ctxT[:, h, ig * 128:(ig + 1) * 128],
                                pst[:], AF.Copy)
                    # o_proj for this i-block (all 4 heads ready)
                    for it in range(4):
                        itg = ib * 4 + it
                        for nb in range(HID // 512):
                            pso = pp3.tile([128, 512], F32, tag="pso")
                            for c4 in range(QH):
                                nc.tensor.matmul(
                                    pso[:],
                                    ctxT[:, c4, itg * 128:(itg + 1) * 128],
                                    wo_sb[:, c4, nb * 512:(nb + 1) * 512],
                                    start=(c4 == 0), stop=(c4 == QH - 1))
                            o_sb = p3d.tile([128, 512], F32, tag="osb")
                            if nb % 2 == 0:
                                nc.vector.tensor_copy(o_sb[:], pso[:])
                            else:
                                nc.scalar.activation(o_sb[:], pso[:], AF.Copy)
                            nc.sync.dma_start(
                                out[itg * 128:(itg + 1) * 128,
                                    nb * 512:(nb + 1) * 512], o_sb[:])

    nc.compile()
    return nc


_CACHE: dict = {}


def _get_program(mask_mode: str):
    if mask_mode not in _CACHE:
        _CACHE[mask_mode] = build_program(mask_mode)
    return _CACHE[mask_mode]


def _host_tensors():
    """Position-dependent constants shared by every call."""
    inv_freq = 1.0 / (THETA ** (np.arange(0, HD, 2, dtype=np.float32) / HD))
    t = np.arange(S, dtype=np.float32)
    freqs = np.outer(t, inv_freq)                     # [S, 64]
    emb = np.concatenate([freqs, freqs], axis=-1)     # [S, 128]
    cosT = np.cos(emb).T.astype(np.float32).copy()    # [128, S]
    sinT = np.sin(emb).T.astype(np.float32).copy()
    sinT[0:64] *= -1.0                                # fold rotate_half sign
    cs = np.ascontiguousarray(np.stack([cosT, sinT]))  # [2, 128, S]
    idm = np.eye(128, dtype=np.float32)
    jj = np.arange(128)[:, None]
    ii = np.arange(512)[None, :]
    stair = np.stack([np.where(ii >= 128 * r + jj, 1.0, 0.0)
                      for r in range(4)]).astype(bfloat16)  # [4, 128, 512]
    return cs, idm, stair


def kernel(hidden_states, Wq, Wk, Wv, Wo, attention_mask):
    hidden_states = np.asarray(hidden_states, dtype=np.float32)
    Wq = np.asarray(Wq, dtype=np.float32)
    Wk = np.asarray(Wk, dtype=np.float32)
    Wv = np.asarray(Wv, dtype=np.float32)
    Wo = np.asarray(Wo, dtype=np.float32)
    mask = np.asarray(attention_mask, dtype=np.float32)[0, 0]

    causal_ref = np.triu(np.full((S, S), NEG, dtype=np.float32), k=1)
    if np.array_equal(mask, causal_ref):
        mode = "causal"
    elif not mask.any():
        mode = "none"
    else:
        mode = "full"

    nc = _get_program(mode)
    cs, idm, stair = _host_tensors()

    hT = np.ascontiguousarray(hidden_states[0].T).astype(bfloat16)
    wq_b = Wq.astype(bfloat16)
    wk_b = Wk.astype(bfloat16)
    wv_b = Wv.astype(bfloat16)
    wo_b = Wo.astype(bfloat16)

    in_maps = []
    for c in range(NCORES):
        m = {
            "hT": hT,
            "wq": np.ascontiguousarray(wq_b[:, c * DQ:(c + 1) * DQ]),
            "wk": np.ascontiguousarray(wk_b[:, c * HD:(c + 1) * HD]),
            "wv": np.ascontiguousarray(wv_b[:, c * HD:(c + 1) * HD]),
            "wo": np.ascontiguousarray(wo_b[c * DQ:(c + 1) * DQ, :]),
            "cs": cs,
            "idm": idm,
        }
        if mode == "causal":
            m["stair"] = stair
        if mode == "full":
            m["maskT"] = np.ascontiguousarray(mask.T * math.sqrt(HD)).astype(
                np.float32)
        in_maps.append(m)

    res = run_bass_kernel_spmd(nc, in_maps, core_ids=list(range(NCORES)))
    total = res.results[0]["out"].astype(np.float32)
    for c in range(1, NCORES):
        total = total + res.results[c]["out"]
    return total.reshape(B, S, HID).astype(np.float32)


# revision 12
# speedup vs baseline: 1.1695x; 1.0020x over previous
"""Tensor-parallel LLaMA attention (B=1, S=2048, HID=4096, 32 Q heads / 8 KV
heads, HD=128) on 8 TRN2 NeuronCores.

Sharding: core c owns Q heads [4c..4c+3] and KV head c (column-parallel
q/k/v_proj, row-parallel o_proj). Each core emits a partial [S, HID] output;
the host sums the 8 partials (the all-reduce of the row-parallel o_proj).

Per-core kernel layout strategy (everything [partition, free]):
  - qT/kT produced directly in [d, s] layout (weights as matmul stationary),
    RoPE applied in that layout via partition-offset reads.
  - v produced in natural [s, d] layout with a ones column appended ([s, 129])
    so the ctx matmul's extra column accumulates the softmax row-sum for free.
  - scores computed transposed sT[j, i] = k @ q^T; softmax = exp (no max pass:
    inputs are unit-variance so scores are O(1)); normalization deferred to a
    per-partition scalar multiply after the ctx matmul.
  - ctx[i, 129] -> normalize -> PE-transpose -> ctxT[d, i] -> o_proj.
Causal masking is structural (upper-triangle blocks skipped; diagonal blocks
get a precomputed staircase mask added pre-exp).
"""

import math
import numpy as np
from ml_dtypes import bfloat16

import concourse.bass as bass
import concourse.bacc as bacc
import concourse.tile as tile
import concourse.mybir as mybir
from concourse.bass_utils import run_bass_kernel_spmd

F32 = mybir.dt.float32
BF16 = mybir.dt.bfloat16
AF = mybir.ActivationFunctionType

B, S, HID = 1, 2048, 4096
NH, NKV, HD = 32, 8, 128
NCORES = 8
QH = NH // NCORES          # 4 q heads per core
DQ = QH * HD               # 512
KC = HID // 128            # 32 contraction chunks
NT = S // 128              # 16 s-tiles
NB = S // 512              # 4 s-blocks
THETA = 10000.0
SCALE = 1.0 / math.sqrt(HD)
NEG = -1.0e9


def build_program(mask_mode: str):
    """mask_mode: 'causal' | 'none' | 'full'"""
    nc = bacc.Bacc("TRN2", target_bir_lowering=False, debug=False,
                   enable_asserts=False, num_devices=NCORES)

    hT = nc.dram_tensor("hT", [HID, S], BF16, kind="ExternalInput")
    wq = nc.dram_tensor("wq", [HID, DQ], BF16, kind="ExternalInput")
    wk = nc.dram_tensor("wk", [HID, HD], BF16, kind="ExternalInput")
    wv = nc.dram_tensor("wv", [HID, HD], BF16, kind="ExternalInput")
    wo = nc.dram_tensor("wo", [DQ, HID], BF16, kind="ExternalInput")
    cs = nc.dram_tensor("cs", [2, HD, S], F32, kind="ExternalInput")
    idm = nc.dram_tensor("idm", [HD, HD], F32, kind="ExternalInput")
    if mask_mode == "causal":
        stair = nc.dram_tensor("stair", [4, HD, 512], BF16, kind="ExternalInput")
    if mask_mode == "full":
        maskT = nc.dram_tensor("maskT", [S, S], F32, kind="ExternalInput")
    out = nc.dram_tensor("out", [S, HID], F32, kind="ExternalOutput")

    hT_r = hT.rearrange("(c p) s -> p c s", p=128)     # [128, 32, 2048]
    wq_r = wq.rearrange("(c p) m -> p c m", p=128)     # [128, 32, 512]
    wk_r = wk.rearrange("(c p) m -> p c m", p=128)
    wv_r = wv.rearrange("(c p) m -> p c m", p=128)
    wo_r = wo.rearrange("(c p) n -> p c n", p=128)     # [128, 4, 4096]

    with tile.TileContext(nc) as tc:
        with tc.tile_pool(name="persist", bufs=1) as pers:
            qT4 = pers.tile([128, QH, S], BF16)
            kT = pers.tile([128, S], BF16)
            vh = pers.tile([128, NT, 132], BF16)
            ctxT = pers.tile([128, QH, S], BF16)
            id_sb = pers.tile([128, 128], F32)
            nc.sync.dma_start(id_sb[:], idm[:])
            if mask_mode == "causal":
                stair_sb = pers.tile([128, 4, 512], BF16)
                nc.sync.dma_start(stair_sb[:], stair[:].rearrange("r p f -> p r f"))

            # ---------------- Phase 1: QKV projection + RoPE ----------------
            with tc.tile_pool(name="ph1", bufs=1) as p1, \
                 tc.tile_pool(name="ph1d", bufs=2) as p1d, \
                 tc.tile_pool(name="pp1", bufs=2, space="PSUM") as pp1:
                # DMA order = first-needed-first: the opening q-chain reads
                # wq chunk 0 + ht chunk 0, so interleave those streams; k/v
                # weights and cos/sin aren't needed until ~15us in.
                wq_sb = p1.tile([128, KC, DQ], BF16)
                wk_sb = p1.tile([128, KC, HD], BF16)
                wv_sb = p1.tile([128, KC, HD], BF16)
                cos_sb = p1.tile([128, S], F32)
                sin_sb = p1.tile([128, S], F32)
                ht0 = p1d.tile([128, KC, 512], BF16, tag="ht")
                nc.sync.dma_start(wq_sb[:, 0:2, :], wq_r[:, 0:2, :])
                nc.sync.dma_start(ht0[:, 0:2, :], hT_r[:, 0:2, 0:512])
                nc.sync.dma_start(wq_sb[:, 2:8, :], wq_r[:, 2:8, :])
                nc.sync.dma_start(ht0[:, 2:8, :], hT_r[:, 2:8, 0:512])
                nc.sync.dma_start(wq_sb[:, 8:16, :], wq_r[:, 8:16, :])
                nc.sync.dma_start(ht0[:, 8:16, :], hT_r[:, 8:16, 0:512])
                nc.sync.dma_start(wq_sb[:, 16:24, :], wq_r[:, 16:24, :])
                nc.sync.dma_start(ht0[:, 16:24, :], hT_r[:, 16:24, 0:512])
                nc.sync.dma_start(wq_sb[:, 24:32, :], wq_r[:, 24:32, :])
                nc.sync.dma_start(ht0[:, 24:32, :], hT_r[:, 24:32, 0:512])
                nc.sync.dma_start(cos_sb[:, 0:512], cs[0, :, 0:512])
                nc.sync.dma_start(sin_sb[:, 0:512], cs[1, :, 0:512])
                nc.sync.dma_start(cos_sb[:, 512:], cs[0, :, 512:])
                nc.sync.dma_start(sin_sb[:, 512:], cs[1, :, 512:])
                nc.sync.dma_start(wk_sb[:], wk_r[:])
                nc.sync.dma_start(wv_sb[:], wv_r[:])

                for st in range(NT):
                    nc.vector.memset(vh[:, st, 128:129], 1.0)

                for sb in range(NB):
                    sl = slice(sb * 512, (sb + 1) * 512)
                    if sb == 0:
                        ht = ht0
                    else:
                        ht = p1d.tile([128, KC, 512], BF16, tag="ht")
                        nc.sync.dma_start(ht[:, 0:8, :], hT_r[:, 0:8, sl])
                        nc.sync.dma_start(ht[:, 8:16, :], hT_r[:, 8:16, sl])
                        nc.sync.dma_start(ht[:, 16:24, :], hT_r[:, 16:24, sl])
                        nc.sync.dma_start(ht[:, 24:32, :], hT_r[:, 24:32, sl])
                    # q (4 head-tiles) then k
                    for hti in range(QH + 1):
                        ps = pp1.tile([128, 512], F32, tag="ps1", bufs=4)
                        for c in range(KC):
                            lhsT = (wq_sb[:, c, hti * 128:(hti + 1) * 128]
                                    if hti < QH else wk_sb[:, c, :])
                            nc.tensor.matmul(ps[:], lhsT, ht[:, c, :],
                                             start=(c == 0), stop=(c == KC - 1))
                        # RoPE in [d, s] layout
                        tc_ = p1d.tile([128, 512], F32, tag="tcos")
                        ts_ = p1d.tile([128, 512], F32, tag="tsin")
                        nc.vector.tensor_mul(tc_[:], ps[:], cos_sb[:, sl])
                        nc.vector.tensor_mul(ts_[0:64, :], ps[64:128, :],
                                             sin_sb[0:64, sl])
                        nc.vector.tensor_mul(ts_[64:128, :], ps[0:64, :],
                                             sin_sb[64:128, sl])
                        dest = (qT4[:, hti, sl] if hti < QH else kT[:, sl])
                        nc.vector.tensor_add(dest, tc_[:], ts_[:])
                    # v in [s, d] layout (+ ones col already set)
                    for st4 in range(4):
                        st = sb * 4 + st4
                        psv = pp1.tile([128, 128], F32, tag="psv")
                        for c in range(KC):
                            nc.tensor.matmul(psv[:],
                                             ht[:, c, st4 * 128:(st4 + 1) * 128],
                                             wv_sb[:, c, :],
                                             start=(c == 0), stop=(c == KC - 1))
                        nc.scalar.activation(vh[:, st, 0:128], psv[:], AF.Copy)

            # ------- Phase 2+3: attention with interleaved O projection ------
            # i-block outer so o_proj for block ib (needing all 4 heads'
            # ctxT columns) interleaves with attention of block ib+1,
            # keeping PE fed through the ACT/DVE softmax chains.
            with tc.tile_pool(name="ph2", bufs=2) as p2, \
                 tc.tile_pool(name="ph3", bufs=1) as p3, \
                 tc.tile_pool(name="ph3d", bufs=4) as p3d, \
                 tc.tile_pool(name="pp2", bufs=2, space="PSUM") as pp2, \
                 tc.tile_pool(name="pp2c", bufs=2, space="PSUM") as pp2c, \
                 tc.tile_pool(name="pp3", bufs=2, space="PSUM") as pp3:
                wo_sb = p3.tile([128, QH, HID], BF16)
                nc.sync.dma_start(wo_sb[:, 0:2, :], wo_r[:, 0:2, :])
                nc.sync.dma_start(wo_sb[:, 2:4, :], wo_r[:, 2:4, :])
                for ib in range(NB):
                    isl = slice(ib * 512, (ib + 1) * 512)
                    jmax = 4 * (ib + 1) if mask_mode == "causal" else NT
                    for h in range(QH):
                        pT = p2.tile([128, NT, 512], BF16, tag="pT")
                        for jt in range(jmax):
                            pss = pp2.tile([128, 512], F32, tag="pss")
                            nc.tensor.matmul(pss[:],
                                             kT[:, jt * 128:(jt + 1) * 128],
                                             qT4[:, h, isl],
                                             start=True, stop=True)
                            if mask_mode == "full":
                                mt = p2.tile([128, 512], F32, tag="mt")
                                nc.sync.dma_start(
                                    mt[:], maskT[jt * 128:(jt + 1) * 128, isl])
                                nc.vector.tensor_add(pss[:], pss[:], mt[:])
                            nc.scalar.activation(pT[:, jt, :], pss[:], AF.Exp,
                                                 scale=SCALE)
                            if mask_mode == "causal" and jt >= 4 * ib:
                                # zero the upper triangle post-exp (0/1 mask,
                                # bf16 SBUF in-place: 4x DVE rate, off PSUM)
                                r = jt - 4 * ib
                                nc.vector.tensor_mul(pT[:, jt, :],
                                                     pT[:, jt, :],
                                                     stair_sb[:, r, :])
                        for it in range(4):
                            ig = ib * 4 + it
                            jm = ig + 1 if mask_mode == "causal" else NT
                            psc = pp2c.tile([128, 132], F32, tag="psc")
                            for jt in range(jm):
                                nc.tensor.matmul(
                                    psc[:, 0:129],
                                    pT[:, jt, it * 128:(it + 1) * 128],
                                    vh[:, jt, 0:129],
                                    start=(jt == 0), stop=(jt == jm - 1))
                            rec = p2.tile([128, 1], F32, tag="rec")
                            nc.vector.reciprocal(rec[:], psc[:, 128:129])
                            cn = p2.tile([128, 128], F32, tag="cn")
                            nc.vector.tensor_scalar_mul(cn[:], psc[:, 0:128],
                                                        rec[:])
                            pst = pp2c.tile([128, 128], F32, tag="pst", bufs=2)
                            nc.tensor.transpose(pst[:], cn[:], id_sb[:])
                            nc.scalar.activation(
                                ctxT[:, h, ig * 128:(ig + 1) * 128],
                                pst[:], AF.Copy)
                    # o_proj for this i-block (all 4 heads ready)
                    for it in range(4):
                        itg = ib * 4 + it
                        for nb in range(HID // 512):
                            pso = pp3.tile([128, 512], F32, tag="pso")
                            for c4 in range(QH):
                                nc.tensor.matmul(
                                    pso[:],
                                    ctxT[:, c4, itg * 128:(itg + 1) * 128],
                                    wo_sb[:, c4, nb * 512:(nb + 1) * 512],
                                    start=(c4 == 0), stop=(c4 == QH - 1))
                            o_sb = p3d.tile([128, 512], F32, tag="osb")
                            if nb % 2 == 0:
                                nc.vector.tensor_copy(o_sb[:], pso[:])
                            else:
                                nc.scalar.activation(o_sb[:], pso[:], AF.Copy)
                            nc.sync.dma_start(
                                out[itg * 128:(itg + 1) * 128,
                                    nb * 512:(nb + 1) * 512], o_sb[:])

    nc.compile()
    return nc


_CACHE: dict = {}


def _get_program(mask_mode: str):
    if mask_mode not in _CACHE:
        _CACHE[mask_mode] = build_program(mask_mode)
    return _CACHE[mask_mode]


def _host_tensors():
    """Position-dependent constants shared by every call."""
    inv_freq = 1.0 / (THETA ** (np.arange(0, HD, 2, dtype=np.float32) / HD))
    t = np.arange(S, dtype=np.float32)
    freqs = np.outer(t, inv_freq)                     # [S, 64]
    emb = np.concatenate([freqs, freqs], axis=-1)     # [S, 128]
    cosT = np.cos(emb).T.astype(np.float32).copy()    # [128, S]
    sinT = np.sin(emb).T.astype(np.float32).copy()
    sinT[0:64] *= -1.0                                # fold rotate_half sign
    cs = np.ascontiguousarray(np.stack([cosT, sinT]))  # [2, 128, S]
    idm = np.eye(128, dtype=np.float32)
    jj = np.arange(128)[:, None]
    ii = np.arange(512)[None, :]
    stair = np.stack([np.where(ii >= 128 * r + jj, 1.0, 0.0)
                      for r in range(4)]).astype(bfloat16)  # [4, 128, 512]
    return cs, idm, stair


def kernel(hidden_states, Wq, Wk, Wv, Wo, attention_mask):
    hidden_states = np.asarray(hidden_states, dtype=np.float32)
    Wq = np.asarray(Wq, dtype=np.float32)
    Wk = np.asarray(Wk, dtype=np.float32)
    Wv = np.asarray(Wv, dtype=np.float32)
    Wo = np.asarray(Wo, dtype=np.float32)
    mask = np.asarray(attention_mask, dtype=np.float32)[0, 0]

    causal_ref = np.triu(np.full((S, S), NEG, dtype=np.float32), k=1)
    if np.array_equal(mask, causal_ref):
        mode = "causal"
    elif not mask.any():
        mode = "none"
    else:
        mode = "full"

    nc = _get_program(mode)
    cs, idm, stair = _host_tensors()

    hT = np.ascontiguousarray(hidden_states[0].T).astype(bfloat16)
    wq_b = Wq.astype(bfloat16)
    wk_b = Wk.astype(bfloat16)
    wv_b = Wv.astype(bfloat16)
    wo_b = Wo.astype(bfloat16)

    in_maps = []
    for c in range(NCORES):
        m = {
            "hT": hT,
            "wq": np.ascontiguousarray(wq_b[:, c * DQ:(c + 1) * DQ]),
            "wk": np.ascontiguousarray(wk_b[:, c * HD:(c + 1) * HD]),
            "wv": np.ascontiguousarray(wv_b[:, c * HD:(c + 1) * HD]),
            "wo": np.ascontiguousarray(wo_b[c * DQ:(c + 1) * DQ, :]),
            "cs": cs,
            "idm": idm,
        }
        if mode == "causal":
            m["stair"] = stair
        if mode == "full":
            m["maskT"] = np.ascontiguousarray(mask.T * math.sqrt(HD)).astype(
                np.float32)
        in_maps.append(m)

    res = run_bass_kernel_spmd(nc, in_maps, core_ids=list(range(NCORES)))
    total = res.results[0]["out"].astype(np.float32)
    for c in range(1, NCORES):
        total = total + res.results[c]["out"]
    return total.reshape(B, S, HID).astype(np.float32)
